# revision 25
# baseline (speedup 1.0000x reference)
"""Trainium2 Bass kernel for attention energies + softmax.

Computes: energies = encoder_outputs[8192,4096] @ hidden[4096] ; softmax -> [1,1,8192]

Sharding: encoder_outputs split along seq_len across 8 NeuronCores
(1024 rows each). Each core streams its 16 MiB shard from HBM into
SBUF and computes local energies with fused multiply+accumulate
(scalar_tensor_tensor) on the DVE. The softmax is emitted in
shard-local form and combined exactly during the host-side gather
with the standard log-sum-exp rescale over per-(core,partition)
groups: group A = tiles 0..5 with reference mA[p] = max_t e[p,t]
(numerators exp(e - mA) <= 1, no overflow possible); tiles 6 and 7
are their own reference (numerator identically 1), so the device
ships their raw energies and computes no exp for them — this removes
every data dependency between the last-arriving tiles and the exp
reference, keeping all reduces off the end-of-stream critical path.
out[group] = n * exp(m_group - M) / S with M, S reduced on host in
fp64.

Key structure (from perfetto/NTFF analysis on trn2):
- No collectives. The ncfw collective path costs a fixed ~61 us
  firmware-boot barrier + ~11 us cold first-dispatch + ~15 us of
  serialized warmup+AllGather before a 32 B stats exchange can
  complete (measured 97 us total vs ~51-64 us for the last HBM
  byte). Combining at gather time removes that entire tail; the
  kernel is HBM-stream-bound end to end.
- No gpsimd. The Q7 cores pay a ~6 us IRAM ucode load on first use
  of each custom op, which gated the DVE start at ~19 us in earlier
  revisions. The h broadcast to 128 partitions is done by the DMA
  engines (stride-0 partition-broadcast AP from DRAM) on the scalar
  HWDGE ring, concurrent with the eo stream on the sync ring; the
  per-partition exp reference needs no cross-partition reduction.
- The eo stream runs at ~300-400 GB/s (run-to-run DVFS/HBM
  variance). The stream tapers: tiles 0..5 as 1 MiB halves (DVE stt
  2.29 us vs ~2.7-3.3 us arrival per half), tiles 6..7 as 512 KiB
  quarters with the last two chunks 256 KiB eighths, so the last
  multiply trails the last HBM byte by well under 1 us. The e05
  add / mA max / -mA / e6 reduce all execute in the DVE's
  DMA-wait gaps mid-stream; the only post-stream DVE work is the
  final chunk's stt and the e7 reduce.
- A DVE memset+stt warmup absorbs the ~2.8 us first-stt penalty.
- Single output [P, 16] (64 B-per-partition rows; a [P, 1] output
  was measured to cost ~6 us extra in DMA completion receipts from
  4 B HBM read-modify-write descriptors): cols 0..5 group-A
  numerators, col 6 mA, col 7 raw e6, col 8 raw e7, cols 9..15 pad.
"""

from contextlib import ExitStack

import numpy as np

import concourse.bacc as bacc
import concourse.tile as tile
from concourse import mybir
from concourse.bass_utils import run_bass_kernel_spmd

P = 128          # SBUF partitions
H = 4096         # hidden dim
S = 8192         # full seq len
NCORES = 8
SL = S // NCORES  # 1024 rows per core
T = SL // P       # 8 seq tiles per core
HH = H // 2      # half hidden
HQ = H // 4      # quarter hidden
HE = H // 8      # eighth hidden
OC = 16          # output columns (64 B rows)

F32 = mybir.dt.float32
AX = mybir.AxisListType
OP = mybir.AluOpType
ACT = mybir.ActivationFunctionType

# chunk splits (column offset, width)
CH_HALVES = [(0, HH), (HH, HH)]
CH_QUARTERS = [(0, HQ), (HQ, HQ), (2 * HQ, HQ), (3 * HQ, HQ)]
CH_TAPER = [(0, HQ), (HQ, HQ), (2 * HQ, HQ), (3 * HQ, HE), (3 * HQ + HE, HE)]


def build_kernel():
    nc = bacc.Bacc(
        "TRN2",
        target_bir_lowering=False,
        debug=False,
        num_devices=NCORES,
    )
    hidden_d = nc.dram_tensor("hidden", [1, H], F32, kind="ExternalInput").ap()
    eo_d = nc.dram_tensor("eo", [SL, H], F32, kind="ExternalInput").ap()
    outA_d = nc.dram_tensor("outA", [P, 8], F32, kind="ExternalOutput").ap()
    outB6_d = nc.dram_tensor("outB6", [P, 8], F32, kind="ExternalOutput").ap()
    outB7_d = nc.dram_tensor("outB7", [P, 8], F32, kind="ExternalOutput").ap()

    eo_t = eo_d.rearrange("(t p) h -> t p h", p=P)

    with tile.TileContext(nc) as tc, ExitStack() as ctx:
        sb = ctx.enter_context(tc.tile_pool(name="sb", bufs=1))

        # ---- tiles ----
        h_sbA = sb.tile([P, HH], F32)   # h[0:2048] on all partitions
        h_sbB = sb.tile([P, HH], F32)   # h[2048:4096] on all partitions
        eo_sb = [
            sb.tile([P, H], F32, name=f"eo{t}") for t in range(T)
        ]
        scrA = sb.tile([P, HH], F32)    # stt dummy out
        eA5 = sb.tile([P, T - 2], F32)  # tiles 0..5, low-H partial dots
        eB5 = sb.tile([P, T - 2], F32)  # tiles 0..5, high-H partial dots
        e6q = sb.tile([P, 8], F32)      # tile 6 quarter partials (cols 0..3)
        e7q = sb.tile([P, 8], F32)      # tile 7 chunk partials (cols 0..4)
        e_05 = sb.tile([P, T - 2], F32)  # energies, tiles 0..5
        nmb = sb.tile([P, 1], F32)      # -mA
        oA = sb.tile([P, 8], F32)       # cols 0..5 numerators, col 6 mA
        wrm = sb.tile([P, 8], F32)      # DVE warmup scratch
        wrm2 = sb.tile([P, 8], F32)
        wacc = sb.tile([P, 1], F32)

        def stt(in0, in1, acc, w):
            nc.vector.scalar_tensor_tensor(
                out=scrA[:, 0:w], in0=in0, scalar=1.0, in1=in1,
                op0=OP.mult, op1=OP.mult, accum_out=acc,
            )

        def h_slice(off, w):
            if off < HH:
                return h_sbA[:, off : off + w]
            return h_sbB[:, off - HH : off - HH + w]

        # ---- warmups (absorb DVE first-stt ucode penalty; zero the
        # output pad columns) ----
        nc.vector.memset(wrm[:], 0.0)
        nc.vector.memset(oA[:, 7:8], 0.0)
        nc.vector.memset(e6q[:, 4:8], 0.0)
        nc.vector.memset(e7q[:, 5:8], 0.0)
        nc.vector.scalar_tensor_tensor(
            out=wrm2[:], in0=wrm[:], scalar=1.0, in1=wrm[:],
            op0=OP.mult, op1=OP.mult, accum_out=wacc[:],
        )

        # ---- startup ----
        # the tapered eo stream on the sync queue. The A-halves of
        # tiles 0..2 are loaded (and consumed) before any B-half so
        # the DVE never stalls on the second h broadcast (h_sbB lands
        # ~23.6 us; h_sbA ~17.4 us).
        half_order = [(0, 0), (1, 0), (2, 0), (0, 1), (1, 1), (2, 1)]
        half_order += [(t, i) for t in range(3, T - 2) for i in (0, 1)]
        for t, i in half_order:
            off, w = CH_HALVES[i]
            nc.sync.dma_start(
                out=eo_sb[t][:, off : off + w],
                in_=eo_t[t, :, off : off + w],
            )
        for off, w in CH_TAPER:
            nc.sync.dma_start(
                out=eo_sb[T - 1][:, off : off + w],
                in_=eo_t[T - 1, :, off : off + w],
            )

        # h replicated to all 128 partitions by the DMA engines on the
        # scalar HWDGE ring (stride-0 partition-broadcast AP reading
        # the same 8 KiB of DRAM per partition; SBUF sources reject
        # zero partition step), concurrent with the eo stream.
        nc.scalar.dma_start(
            out=h_sbA[:], in_=hidden_d[:, 0:HH].partition_broadcast(P)
        )
        nc.scalar.dma_start(
            out=h_sbB[:], in_=hidden_d[:, HH:H].partition_broadcast(P)
        )
        # tile 6 rides the otherwise-idle scalar ring: it is consumed
        # near the end of the stream but lands by ~35 us, shortening
        # the sync ring's stream by 2 MiB with no arrival-order risk.
        for off, w in CH_QUARTERS:
            nc.scalar.dma_start(
                out=eo_sb[T - 2][:, off : off + w],
                in_=eo_t[T - 2, :, off : off + w],
            )

        # ---- local energies (fused mult+accum on DVE) ----
        # tile 6's quarter stts (data early via the scalar ring) fill
        # the DVE's DMA-wait gaps between the tile 3..4 half-stts, so
        # the only compute left after tile 5 is the A-chain and the
        # tile 7 taper.
        t6_after = {(3, 0): 0, (3, 1): 1, (4, 0): 2, (4, 1): 3}
        for t, i in half_order:
            if i == 0:
                stt(eo_sb[t][:, 0:HH], h_sbA[:], eA5[:, t : t + 1], HH)
            else:
                stt(eo_sb[t][:, HH:H], h_sbB[:], eB5[:, t : t + 1], HH)
            if (t, i) in t6_after:
                qi = t6_after[(t, i)]
                off, w = CH_QUARTERS[qi]
                stt(eo_sb[T - 2][:, off : off + w], h_slice(off, w),
                    e6q[:, qi : qi + 1], w)

        # group-A energies + reference; the exp runs on the scalar
        # engine while tiles 6..7 still stream, and the DVE reduces
        # hide in its DMA-wait gaps
        nc.vector.tensor_tensor(out=e_05[:], in0=eA5[:], in1=eB5[:], op=OP.add)
        nc.vector.tensor_reduce(
            out=oA[:, 6:7], in_=e_05[:], axis=AX.X, op=OP.max
        )
        nc.vector.tensor_scalar_mul(nmb[:], oA[:, 6:7], -1.0)
        nc.scalar.activation(
            oA[:, 0 : T - 2], e_05[:], ACT.Exp, bias=nmb[:], scale=1.0
        )
        nc.scalar.dma_start(out=outA_d, in_=oA[:])

        # tile 6's raw partials ship as soon as its last gap-filled
        # stt retires (mid-stream, hidden)
        nc.scalar.dma_start(out=outB6_d, in_=e6q[:])

        # tile 7 tapered chunks; raw energy, its own reference
        for i, (off, w) in enumerate(CH_TAPER):
            stt(eo_sb[T - 1][:, off : off + w], h_slice(off, w),
                e7q[:, i : i + 1], w)
        nc.scalar.dma_start(out=outB7_d, in_=e7q[:])

    nc.compile()
    return nc


_NC = None


def _get_nc():
    global _NC
    if _NC is None:
        _NC = build_kernel()
    return _NC


def _make_in_maps(hidden: np.ndarray, encoder_outputs: np.ndarray):
    hidden = np.ascontiguousarray(np.asarray(hidden, dtype=np.float32)).reshape(1, H)
    eo = np.ascontiguousarray(np.asarray(encoder_outputs, dtype=np.float32))
    assert eo.shape == (S, H), eo.shape
    return [
        {"hidden": hidden, "eo": eo[c * SL : (c + 1) * SL]} for c in range(NCORES)
    ]


def _combine(bufs) -> np.ndarray:
    """Host-side softmax combine (exact log-sum-exp over shard groups).

    bufs[c] = (bufA, bufB6, bufB7), each [P, 8]: bufA cols 0..5 the
    tile 0..5 numerators exp(e - mA[p]) and col 6 mA[p]; bufB6/bufB7
    the raw tile 6/7 dot-product partials (4 and 5 of them) summed
    here in fp64 (tiles 6/7 are their own reference; numerator == 1).
    """
    nA = np.empty((NCORES, P, T - 2), dtype=np.float64)
    mA = np.empty((NCORES, P), dtype=np.float64)
    e6 = np.empty((NCORES, P), dtype=np.float64)
    e7 = np.empty((NCORES, P), dtype=np.float64)
    for c, (bufA, bufB6, bufB7) in enumerate(bufs):
        a = np.asarray(bufA, dtype=np.float64).reshape(P, 8)
        nA[c] = a[:, : T - 2]
        mA[c] = a[:, T - 2]
        e6[c] = np.asarray(bufB6, dtype=np.float64).reshape(P, 8)[:, :4].sum(axis=1)
        e7[c] = np.asarray(bufB7, dtype=np.float64).reshape(P, 8)[:, :5].sum(axis=1)
    M = max(mA.max(), e6.max(), e7.max())
    wA = np.exp(mA - M)
    w6 = np.exp(e6 - M)
    w7 = np.exp(e7 - M)
    Ssum = (nA.sum(axis=2) * wA).sum() + w6.sum() + w7.sum()
    out = np.empty((NCORES, T, P), dtype=np.float64)
    out[:, : T - 2, :] = (nA * (wA / Ssum)[:, :, None]).transpose(0, 2, 1)
    out[:, T - 2, :] = w6 / Ssum
    out[:, T - 1, :] = w7 / Ssum
    # seq index within a core is t*P + p
    return out.reshape(1, 1, S).astype(np.float32)


def kernel(hidden: np.ndarray, encoder_outputs: np.ndarray) -> np.ndarray:
    nc = _get_nc()
    in_maps = _make_in_maps(hidden, encoder_outputs)
    res = run_bass_kernel_spmd(nc, in_maps, core_ids=list(range(NCORES)))
    return _combine(
        [
            (
                res.results[c]["outA"],
                res.results[c]["outB6"],
                res.results[c]["outB7"],
            )
            for c in range(NCORES)
        ]
    )


if __name__ == "__main__":
    rng = np.random.default_rng(0)
    h = rng.standard_normal((1, H), dtype=np.float32)
    eo = rng.standard_normal((S, H), dtype=np.float32)
    got = kernel(hidden=h, encoder_outputs=eo)
    e = eo.astype(np.float64) @ h.reshape(-1).astype(np.float64)
    e -= e.max()
    p = np.exp(e)
    want = (p / p.sum()).reshape(1, 1, S)
    err = np.abs(got.astype(np.float64) - want)
    rel = err.max() / np.abs(want).max()
    print("max abs err:", err.max(), "rel:", rel)
